# revision 19
# baseline (speedup 1.0000x reference)
"""Trainium2 Bass kernel for nn_EnhanceDiversityFeatureExtracition.

Computes  loss = mean((x-y)^2) + ALPHA * diversity_reg(conv_w)
where diversity_reg builds a 64x64 Gram matrix of the F=64 slices
conv_w[:, :, i, :] (each flattened to a 786432-vector), normalizes it to
cosine similarities, and sums the entries with tau < sim <= 1 off the
diagonal.

Distribution (8 NeuronCores, SPMD):
  - x_batch / y_batch sharded on batch dim: 256 rows per core.
  - conv_w viewed as A = conv_w.reshape(262144, 192)  (row m = (o,c),
    col g = f*3+k).  gram[i,j] = sum_k C[3i+k, 3j+k] where C = A^T A,
    so A is sharded along the 262144-row reduction axis: 32768 rows per
    core.  Each core returns its partial C (rows 0:128 and 128:192) and
    per-partition partial sums of (x-y)^2; the host sums the partials,
    extracts the 64x64 gram from C, and applies the tiny masked
    similarity epilogue.

On-core dataflow (DMA-bound; ~32MB of HBM reads per core at ~420GB/s):
  - A shard is streamed in 32 blocks of 1024 rows laid out as
    [128 partitions x 1536 floats] (per-partition contiguous 6KB HWDGE
    DMA).  The A pool is deep (24 bufs) so DMA buffer recycling never
    waits on the PE.
  - Each 128-row tile yields 2 fp32r matmuls (contraction 128,
    stationary cols 0:128 / 128:192, moving width 256 for the fp32r
    full-rate mode) accumulating into two PSUM tiles across the whole
    shard.  The 64 pad columns past each block's 1536 data floats are
    never initialized: they only feed PSUM columns 192:255, which are
    never read.
  - MSE: 4 chunks of [128 x 2048] per operand; DVE computes d = x-y,
    ACT computes Square(d) with a per-partition accumulate; the
    accumulator is written out mid-kernel, off the critical tail.
"""

import numpy as np

import concourse.bass as bass
import concourse.mybir as mybir
from concourse import bacc, tile
from concourse.bass_utils import run_bass_kernel_spmd

N_CORES = 8
B, D = 2048, 4096            # x_batch / y_batch
M, G = 262144, 192           # conv_w as (M, G); G = F*KW
F, KW = 64, 3
ROWS = B // N_CORES          # 256 batch rows per core
MC = M // N_CORES            # 32768 reduction rows per core
TPB = 8                      # 128-row tiles per DMA block
BLK = 128 * TPB              # 1024 rows per block
NBLK = MC // BLK             # 32
NCH = 8                      # MSE chunks per core
CHW = (ROWS * D) // (128 * NCH)  # 1024 floats per partition per chunk
MSE_AT = 2                   # first A block to interleave MSE pieces at

ALPHA = 0.0005
TAU = 0.2

_prog = None


def _build() -> bass.Bass:
    nc = bacc.Bacc(None, target_bir_lowering=False)
    f32 = mybir.dt.float32
    bf16 = mybir.dt.bfloat16

    xs = nc.dram_tensor("xs", [ROWS, D], f32, kind="ExternalInput")
    ys = nc.dram_tensor("ys", [ROWS, D], f32, kind="ExternalInput")
    aw = nc.dram_tensor("aw", [MC, G], f32, kind="ExternalInput")
    c1_part = nc.dram_tensor("c1_part", [128, G], f32, kind="ExternalOutput")
    c2_part = nc.dram_tensor("c2_part", [F, G], f32, kind="ExternalOutput")
    sse_part = nc.dram_tensor("sse_part", [128, NCH], f32, kind="ExternalOutput")

    with tile.TileContext(nc) as tc:
        with (
            tc.tile_pool(name="fpool", bufs=8) as fpool,
            tc.tile_pool(name="apool", bufs=24) as apool,
            tc.tile_pool(name="xpool", bufs=2) as xpool,
            tc.tile_pool(name="ypool", bufs=2) as ypool,
            tc.tile_pool(name="dpool", bufs=2) as dpool,
            tc.tile_pool(name="qpool", bufs=2) as qpool,
            tc.tile_pool(name="opool", bufs=1) as opool,
            tc.tile_pool(name="psum", bufs=1, space=bass.MemorySpace.PSUM) as psum,
        ):
            # C = A^T A accumulators, rows 0-127 and 128-191
            cps1 = psum.tile([128, G], f32, tag="cps1")
            cps2 = psum.tile([F, G], f32, tag="cps2")
            acc = opool.tile([128, NCH], f32)

            # per-partition contiguous views
            xv = xs[:].rearrange("(p t) d -> p (t d)", p=128)
            yv = ys[:].rearrange("(p t) d -> p (t d)", p=128)

            # Warm fp32r matmuls can't keep pace with the 420GB/s DMA
            # stream (the fp32 moving-operand feed is SBUF-bandwidth
            # limited), so cast each block to bf16 on the otherwise-idle
            # DVE and run native full-rate bf16 matmuls at moving
            # width 192.  bf16 rounding is harmless here: sim errors
            # are ~1e-5 against a 0.2 threshold margin.
            #
            # Block sizes in 128-row tiles.  A small first block shortens
            # the time to the first cast/matmul; the last blocks ride the
            # scalar (ACT) HWDGE ring so the two rings' tail completions
            # drain in parallel.  (Mid-stream scalar-ring DMAs would queue
            # behind the MSE ACTIVATE ops, so only the tail goes there.)
            sizes = [4] + [TPB] * 31 + [4]
            n_scalar_tail = 3
            n_t = sum(sizes)
            ti = 0
            si = 0
            tile_base = 0
            for b, sz in enumerate(sizes):
                ft = fpool.tile([128, sz * G], f32)
                eng = nc.scalar if b >= len(sizes) - n_scalar_tail else nc.sync
                src = aw[tile_base * 128:(tile_base + sz) * 128].rearrange(
                    "(p t) g -> p (t g)", p=128
                )
                eng.dma_start(ft[:], src)
                at = apool.tile([128, sz * G], bf16)
                nc.vector.tensor_copy(at[:], ft[:])
                # keep each PSUM accumulation group's matmuls contiguous:
                # ping-ponging groups per instruction breaks MM pipelining
                for t in range(sz):
                    rhs = at[:, t * G:(t + 1) * G]
                    w1 = at[:, t * G:t * G + 128]
                    nc.tensor.matmul(
                        cps1[:], w1, rhs,
                        start=(ti == 0), stop=(ti == n_t - 1),
                    )
                    ti += 1
                for t in range(sz):
                    rhs = at[:, t * G:(t + 1) * G]
                    w2 = at[:, t * G + 128:(t + 1) * G]
                    nc.tensor.matmul(
                        cps2[:], w2, rhs,
                        start=(si == 0), stop=(si == n_t - 1),
                    )
                    si += 1
                tile_base += sz

                # interleave one 512KB MSE piece per A block: keeps the
                # A-block arrival spacing under the ~3.4us HAM idle window
                # so the PE never re-throttles mid-stream
                if MSE_AT <= b < MSE_AT + 2 * NCH:
                    step = b - MSE_AT
                    ch = step // 2
                    if step % 2 == 0:
                        xt = xpool.tile([128, CHW], f32)
                        nc.sync.dma_start(xt[:], xv[:, ch * CHW:(ch + 1) * CHW])
                    else:
                        yt = ypool.tile([128, CHW], f32)
                        nc.sync.dma_start(yt[:], yv[:, ch * CHW:(ch + 1) * CHW])
                        dtile = dpool.tile([128, CHW], f32)
                        nc.vector.tensor_sub(dtile[:], xt[:], yt[:])
                        qtile = qpool.tile([128, CHW], f32)
                        nc.scalar.activation(
                            qtile[:], dtile[:],
                            mybir.ActivationFunctionType.Square,
                            accum_out=acc[:, ch:ch + 1],
                        )
                        if ch == NCH - 1:
                            # SSE done mid-kernel.  Issue its writeback on
                            # the scalar (ACT) HWDGE ring: a sync-ring DMA
                            # here would head-of-line block the whole A
                            # stream behind the MSE compute chain.
                            nc.scalar.dma_start(sse_part[:], acc[:])

            # epilogue: PSUM -> SBUF -> DRAM (gram extraction happens on host)
            csb1 = opool.tile([128, G], f32, tag="csb1")
            nc.vector.tensor_copy(csb1[:], cps1[:])
            nc.sync.dma_start(c1_part[:], csb1[:])
            csb2 = opool.tile([F, G], f32, tag="csb2")
            nc.vector.tensor_copy(csb2[:], cps2[:])
            nc.scalar.dma_start(c2_part[:], csb2[:])

    nc.finalize()
    return nc


def _get_prog() -> bass.Bass:
    global _prog
    if _prog is None:
        _prog = _build()
    return _prog


def _epilogue(C: np.ndarray, sse: float) -> np.ndarray:
    # gram[i,j] = sum_k C[3i+k, 3j+k]
    gram = C[0::KW, 0::KW] + C[1::KW, 1::KW] + C[2::KW, 2::KW]
    norms = np.sqrt(np.diag(gram))
    sim = gram / np.outer(norms, norms)
    mask = (sim > TAU) & (sim <= 1.0) & (~np.eye(F, dtype=bool))
    reg = sim[mask].sum()
    loss = sse / float(B * D) + ALPHA * reg
    return np.asarray(np.float32(loss))


def kernel(x_batch: np.ndarray, y_batch: np.ndarray, conv_w: np.ndarray) -> np.ndarray:
    nc = _get_prog()
    A = np.ascontiguousarray(conv_w.reshape(M, G))
    in_maps = []
    for c in range(N_CORES):
        in_maps.append({
            "xs": np.ascontiguousarray(x_batch[c * ROWS:(c + 1) * ROWS]),
            "ys": np.ascontiguousarray(y_batch[c * ROWS:(c + 1) * ROWS]),
            "aw": np.ascontiguousarray(A[c * MC:(c + 1) * MC]),
        })
    res = run_bass_kernel_spmd(nc, in_maps, core_ids=list(range(N_CORES))).results
    C = np.zeros((G, G), np.float64)
    sse = 0.0
    for r in res:
        C[:128] += r["c1_part"].astype(np.float64)
        C[128:] += r["c2_part"].astype(np.float64)
        sse += float(r["sse_part"].sum(dtype=np.float64))
    return _epilogue(C, sse)


# revision 21
# speedup vs baseline: 1.0184x; 1.0184x over previous
"""Trainium2 Bass kernel for nn_EnhanceDiversityFeatureExtracition.

Computes  loss = mean((x-y)^2) + ALPHA * diversity_reg(conv_w)
where diversity_reg builds a 64x64 Gram matrix of the F=64 slices
conv_w[:, :, i, :] (each flattened to a 786432-vector), normalizes it to
cosine similarities, and sums the entries with tau < sim <= 1 off the
diagonal.

Distribution (8 NeuronCores, SPMD):
  - x_batch / y_batch sharded on batch dim: 256 rows per core.
  - conv_w viewed as A = conv_w.reshape(262144, 192)  (row m = (o,c),
    col g = f*3+k).  gram[i,j] = sum_k C[3i+k, 3j+k] where C = A^T A,
    so A is sharded along the 262144-row reduction axis: 32768 rows per
    core.  Each core returns its partial C (rows 0:128 and 128:192) and
    per-partition partial sums of (x-y)^2; the host sums the partials,
    extracts the 64x64 gram from C, and applies the tiny masked
    similarity epilogue.

On-core dataflow (DMA-bound; ~32MB of HBM reads per core at ~420GB/s):
  - A shard is streamed in 32 blocks of 1024 rows laid out as
    [128 partitions x 1536 floats] (per-partition contiguous 6KB HWDGE
    DMA).  The A pool is deep (24 bufs) so DMA buffer recycling never
    waits on the PE.
  - Each 128-row tile yields 2 fp32r matmuls (contraction 128,
    stationary cols 0:128 / 128:192, moving width 256 for the fp32r
    full-rate mode) accumulating into two PSUM tiles across the whole
    shard.  The 64 pad columns past each block's 1536 data floats are
    never initialized: they only feed PSUM columns 192:255, which are
    never read.
  - MSE: 4 chunks of [128 x 2048] per operand; DVE computes d = x-y,
    ACT computes Square(d) with a per-partition accumulate; the
    accumulator is written out mid-kernel, off the critical tail.
"""

import numpy as np

import concourse.bass as bass
import concourse.mybir as mybir
from concourse import bacc, tile
from concourse.bass_utils import run_bass_kernel_spmd

N_CORES = 8
B, D = 2048, 4096            # x_batch / y_batch
M, G = 262144, 192           # conv_w as (M, G); G = F*KW
F, KW = 64, 3
ROWS = B // N_CORES          # 256 batch rows per core
MC = M // N_CORES            # 32768 reduction rows per core
TPB = 8                      # 128-row tiles per DMA block
BLK = 128 * TPB              # 1024 rows per block
NBLK = MC // BLK             # 32
NCH = 8                      # MSE chunks per core
CHW = (ROWS * D) // (128 * NCH)  # 1024 floats per partition per chunk
MSE_AT = 2                   # first A block to interleave MSE pieces at

ALPHA = 0.0005
TAU = 0.2

_prog = None


def _build() -> bass.Bass:
    nc = bacc.Bacc(None, target_bir_lowering=False)
    f32 = mybir.dt.float32
    bf16 = mybir.dt.bfloat16

    xs = nc.dram_tensor("xs", [ROWS, D], f32, kind="ExternalInput")
    ys = nc.dram_tensor("ys", [ROWS, D], f32, kind="ExternalInput")
    aw = nc.dram_tensor("aw", [MC, G], f32, kind="ExternalInput")
    c1_part = nc.dram_tensor("c1_part", [128, G], f32, kind="ExternalOutput")
    c2_part = nc.dram_tensor("c2_part", [F, G], f32, kind="ExternalOutput")
    sse_part = nc.dram_tensor("sse_part", [128, NCH], f32, kind="ExternalOutput")

    with tile.TileContext(nc) as tc:
        with (
            tc.tile_pool(name="fpool", bufs=8) as fpool,
            tc.tile_pool(name="apool", bufs=NBLK) as apool,
            tc.tile_pool(name="xpool", bufs=2) as xpool,
            tc.tile_pool(name="ypool", bufs=2) as ypool,
            tc.tile_pool(name="dpool", bufs=2) as dpool,
            tc.tile_pool(name="qpool", bufs=2) as qpool,
            tc.tile_pool(name="opool", bufs=1) as opool,
            tc.tile_pool(name="psum", bufs=1, space=bass.MemorySpace.PSUM) as psum,
        ):
            # C = A^T A accumulators, rows 0-127 and 128-191
            cps1 = psum.tile([128, G], f32, tag="cps1")
            cps2 = psum.tile([F, G], f32, tag="cps2")
            acc = opool.tile([128, NCH], f32)

            # per-partition contiguous views
            xv = xs[:].rearrange("(p t) d -> p (t d)", p=128)
            yv = ys[:].rearrange("(p t) d -> p (t d)", p=128)

            # Warm fp32r matmuls can't keep pace with the 420GB/s DMA
            # stream (the fp32 moving-operand feed is SBUF-bandwidth
            # limited), so cast each block to bf16 on the otherwise-idle
            # DVE and run native full-rate bf16 matmuls at moving
            # width 192.  bf16 rounding is harmless here: sim errors
            # are ~1e-5 against a 0.2 threshold margin.
            #
            n_t = NBLK * TPB
            ti = 0
            si = 0
            awv = aw[:].rearrange("(b p t) g -> b p (t g)", p=128, t=TPB)
            for b in range(NBLK):
                ft = fpool.tile([128, TPB * G], f32)
                nc.sync.dma_start(ft[:], awv[b])
                # apool has one buffer per block (no recycling), so the
                # casts depend only on their own DMA completion
                at = apool.tile([128, TPB * G], bf16)
                nc.vector.tensor_copy(at[:], ft[:])
                # keep each PSUM accumulation group's matmuls contiguous:
                # ping-ponging groups per instruction breaks MM pipelining
                for t in range(TPB):
                    rhs = at[:, t * G:(t + 1) * G]
                    w1 = at[:, t * G:t * G + 128]
                    nc.tensor.matmul(
                        cps1[:], w1, rhs,
                        start=(ti == 0), stop=(ti == n_t - 1),
                    )
                    ti += 1
                for t in range(TPB):
                    rhs = at[:, t * G:(t + 1) * G]
                    w2 = at[:, t * G + 128:(t + 1) * G]
                    nc.tensor.matmul(
                        cps2[:], w2, rhs,
                        start=(si == 0), stop=(si == n_t - 1),
                    )
                    si += 1

                # interleave one 512KB MSE piece per A block: keeps the
                # A-block arrival spacing under the ~3.4us HAM idle window
                # so the PE never re-throttles mid-stream
                if MSE_AT <= b < MSE_AT + 2 * NCH:
                    step = b - MSE_AT
                    ch = step // 2
                    if step % 2 == 0:
                        xt = xpool.tile([128, CHW], f32)
                        nc.sync.dma_start(xt[:], xv[:, ch * CHW:(ch + 1) * CHW])
                    else:
                        yt = ypool.tile([128, CHW], f32)
                        nc.sync.dma_start(yt[:], yv[:, ch * CHW:(ch + 1) * CHW])
                        dtile = dpool.tile([128, CHW], f32)
                        nc.vector.tensor_sub(dtile[:], xt[:], yt[:])
                        qtile = qpool.tile([128, CHW], f32)
                        nc.scalar.activation(
                            qtile[:], dtile[:],
                            mybir.ActivationFunctionType.Square,
                            accum_out=acc[:, ch:ch + 1],
                        )
                        if ch == NCH - 1:
                            # SSE done mid-kernel.  Issue its writeback on
                            # the scalar (ACT) HWDGE ring: a sync-ring DMA
                            # here would head-of-line block the whole A
                            # stream behind the MSE compute chain.
                            nc.scalar.dma_start(sse_part[:], acc[:])

            # epilogue: PSUM -> SBUF -> DRAM (gram extraction happens on host)
            csb1 = opool.tile([128, G], f32, tag="csb1")
            nc.vector.tensor_copy(csb1[:], cps1[:])
            nc.sync.dma_start(c1_part[:], csb1[:])
            csb2 = opool.tile([F, G], f32, tag="csb2")
            nc.vector.tensor_copy(csb2[:], cps2[:])
            nc.scalar.dma_start(c2_part[:], csb2[:])

    nc.finalize()
    return nc


def _get_prog() -> bass.Bass:
    global _prog
    if _prog is None:
        _prog = _build()
    return _prog


def _epilogue(C: np.ndarray, sse: float) -> np.ndarray:
    # gram[i,j] = sum_k C[3i+k, 3j+k]
    gram = C[0::KW, 0::KW] + C[1::KW, 1::KW] + C[2::KW, 2::KW]
    norms = np.sqrt(np.diag(gram))
    sim = gram / np.outer(norms, norms)
    mask = (sim > TAU) & (sim <= 1.0) & (~np.eye(F, dtype=bool))
    reg = sim[mask].sum()
    loss = sse / float(B * D) + ALPHA * reg
    return np.asarray(np.float32(loss))


def kernel(x_batch: np.ndarray, y_batch: np.ndarray, conv_w: np.ndarray) -> np.ndarray:
    nc = _get_prog()
    A = np.ascontiguousarray(conv_w.reshape(M, G))
    in_maps = []
    for c in range(N_CORES):
        in_maps.append({
            "xs": np.ascontiguousarray(x_batch[c * ROWS:(c + 1) * ROWS]),
            "ys": np.ascontiguousarray(y_batch[c * ROWS:(c + 1) * ROWS]),
            "aw": np.ascontiguousarray(A[c * MC:(c + 1) * MC]),
        })
    res = run_bass_kernel_spmd(nc, in_maps, core_ids=list(range(N_CORES))).results
    C = np.zeros((G, G), np.float64)
    sse = 0.0
    for r in res:
        C[:128] += r["c1_part"].astype(np.float64)
        C[128:] += r["c2_part"].astype(np.float64)
        sse += float(r["sse_part"].sum(dtype=np.float64))
    return _epilogue(C, sse)
